# revision 1
# baseline (speedup 1.0000x reference)
"""DeepseekV2 MLA (weight-absorbed, chunked-softmax MQA) on 8 trn2 NeuronCores.

Sharding: tensor-parallel over heads (16 heads / 8 cores = 2 heads per core);
the 576-wide latent KV cache is replicated per core. Each core computes its two
heads' full attention output [1024, 256]; the host concatenates along the
feature axis. All matmuls run in bf16 with fp32 PSUM accumulation.

Per-core dataflow (all in transposed [d, t] layouts; no on-chip transposes):
  q_loraT = w_kc.T @ q_nopeT            (PE, per head)         [512, 1024]
  q_fullT = [q_loraT; q_peT]            (SBUF concat)          [576, 1024]
  scoresT = kvT_chunk.T @ q_fullT       (PE, 5 K-chunks)       [128s, 512t]
  pT      = exp(scale * scoresT)        (ACT, PSUM->SBUF bf16)
  attnT  += kv_lora_tile.T @ pT         (PE, accum 64 s-tiles) [128d, 512t] x4
  denom  += ones.T @ pT                 (PE, accum)            [1, 512t]
  denomT  = denom chunks via K=1 matmul (PE)                   [128t, 4]
  out     = attnT.T @ w_vc              (PE, accum 4 d-chunks) [128t, 128v]
  out    *= 1/denomT (per-partition scalar broadcast, DVE), DMA out.
"""

import os
import sys

import numpy as np
import ml_dtypes

for _p in ("/opt/trn_rl_repo",):
    if os.path.isdir(_p) and _p not in sys.path:
        sys.path.append(_p)

import concourse.bass as bass
import concourse.mybir as mybir
import concourse.tile as tile
from concourse.bass_utils import run_bass_kernel_spmd
from concourse.vector_clock import ScopedClock, VectorClock

# ---------------------------------------------------------------- constants
NOPE, ROPE, LORA, VDIM = 128, 64, 512, 128
T, H, S = 1024, 16, 8192
D = LORA + ROPE            # 576 latent dim
SCALING = (NOPE + ROPE) ** -0.5
N_CORES = 8
HPC = H // N_CORES         # heads per core
NST = S // 128             # 64 s-tiles
NTB = T // 512             # 2 t-blocks
NKC = 5                    # K-chunks over 576 = 4*128 + 64
BF16 = mybir.dt.bfloat16
FP32 = mybir.dt.float32
NPBF = ml_dtypes.bfloat16


# ------------------------------------------------- walrus drain workaround
def _patch_tile_drain():
    """The neuronxcc walrus in this container rejects DRAIN instructions
    carrying more than ~2 sync waits ("Too many sync wait commands").
    Split the TileContext exit drain into one drain per processor tick;
    the waits execute sequentially on SP before the all-engine barrier,
    preserving the original semantics."""
    if getattr(tile.TileContext, "_drain_split_patched", False):
        return

    def _drain_and_barrier_split(self, tick_clock, wait_clock):
        gcv = tick_clock.global_clock
        n = len(gcv)
        for proc in range(n):
            t = gcv[proc]
            if t <= 0:
                continue
            vc = VectorClock([0] * n)
            vc.require_at_least(proc, t)
            d = self.nc.sync.drain()
            wait_clock.add_sem_waits(d.ins, ScopedClock({None: vc}))
        self.nc.all_engine_barrier()
        assert self.sems is not None
        popped = self.nc._tile_sem_poison_stack.pop()
        assert popped is self._sem_poison
        self.nc.clear_and_free_semaphores(list(self.sems.allocated().values()))
        self.nc.all_engine_barrier()

    tile.TileContext._drain_and_barrier = _drain_and_barrier_split

    # Same walrus limitation for regular instructions: peel all but the last
    # sync wait off onto same-engine NOPs inserted immediately before the
    # instruction. The engine executes its queue in order, so waiting on the
    # NOPs first is equivalent to one multi-wait instruction.
    orig_add = tile.TileContext._add_instruction

    def _add_instruction_split_waits(self, inst):
        si = inst.sync_info
        if si is not None:
            waits = si.on_wait
            if waits and len(waits) > 1:
                for w in waits[:-1]:
                    nop = mybir.InstNoOp(
                        name=self.nc.get_next_instruction_name(), ins=[], outs=[]
                    )
                    nop.engine = inst.engine
                    nop.sync_info = mybir.SyncInfo(on_wait=[w], on_update=[])
                    orig_add(self, nop)
                inst.sync_info = mybir.SyncInfo(
                    on_wait=[waits[-1]], on_update=si.on_update
                )
        orig_add(self, inst)

    tile.TileContext._add_instruction = _add_instruction_split_waits
    tile.TileContext._drain_split_patched = True


# ------------------------------------------------------------ bass program
MM_KINDS = {}


def _build_program():
    _patch_tile_drain()
    nc = bass.Bass()
    _orig_mm = nc.tensor.matmul

    def _mm_logged(out, lhsT, rhs, kind="?", **kw):
        inst = _orig_mm(out, lhsT, rhs, **kw)
        MM_KINDS[inst.ins.name] = kind
        return inst

    nc.tensor.matmul = _mm_logged
    qnT = nc.declare_dram_parameter("qnT", [HPC, NOPE, T], BF16, isOutput=False)
    qpT = nc.declare_dram_parameter("qpT", [HPC, ROPE, T], BF16, isOutput=False)
    kvT = nc.declare_dram_parameter("kvT", [LORA, S], BF16, isOutput=False)
    kvr = nc.declare_dram_parameter("kvr", [128, S], BF16, isOutput=False)
    kvl = nc.declare_dram_parameter("kvl", [S, LORA], BF16, isOutput=False)
    wkc = nc.declare_dram_parameter("wkc", [HPC, NOPE, LORA], BF16, isOutput=False)
    wvc = nc.declare_dram_parameter("wvc", [HPC, 4, 128, VDIM], BF16, isOutput=False)
    out = nc.declare_dram_parameter("out", [T, HPC * VDIM], FP32, isOutput=True)

    Exp = mybir.ActivationFunctionType.Exp

    with tile.TileContext(nc) as tc:
        with (
            tc.tile_pool(name="res", bufs=1) as res,
            tc.tile_pool(name="kvlp", bufs=12) as kvlp,
            tc.tile_pool(name="ptp", bufs=8) as ptp,
            tc.tile_pool(name="attnsb", bufs=4) as attnsb,
            tc.tile_pool(name="smsb", bufs=4) as smsb,
            tc.tile_pool(name="outsb", bufs=3) as outsb,
            tc.tile_pool(name="ps_sc", bufs=4, space="PSUM") as ps_sc,
            tc.tile_pool(name="ps_attn", bufs=1, space="PSUM") as ps_attn,
        ):
            # ---------------- resident loads (small tensors first: they
            # unblock the q_loraT matmuls that warm up PE while kvT streams)
            qnT_sb = res.tile([NOPE, HPC * T], BF16, tag="qnt")
            wkc_sb = res.tile([NOPE, HPC, LORA], BF16, tag="wkc")
            for h in range(HPC):
                nc.sync.dma_start(qnT_sb[:, h * T:(h + 1) * T], qnT[h])
                nc.sync.dma_start(wkc_sb[:, h, :], wkc[h])
            wvc_sb = res.tile([128, HPC * 4, VDIM], BF16, tag="wvc")
            ones_col = res.tile([128, 1], FP32, tag="ones_col")
            nc.vector.memset(ones_col[:], 1.0)
            ones_f32 = res.tile([1, 1], FP32, tag="ones_f32")
            nc.vector.memset(ones_f32[:], 1.0)

            # PE warmup: ~5us of matmuls on local data so HAM un-throttles and
            # the first real matmuls run at 2.4GHz instead of 1.2
            warm = res.tile([128, 512], BF16, tag="warm")
            nc.vector.memset(warm[:], 0.0)
            wu_ps = ps_sc.tile([128, 512], FP32, tag="sc", name="wu_ps")
            for _w in range(20):
                nc.tensor.matmul(wu_ps[:], warm[:, 0:128], warm[:, 0:512],
                                 kind="warm")


            # ---------------- q_fullT = [w_kc.T @ q_nopeT ; q_peT]  per head
            # rope rows are duplicated to partitions 64:128 so pairs of K=64
            # rope matmuls can run concurrently in disjoint PE row-groups
            kvT_sb = [
                res.tile([128, S], BF16, tag=f"kvt{c}", name=f"kvt{c}")
                for c in range(4)
            ]
            kvr_sb = res.tile([128, S], BF16, tag="kvr")

            def load_kv_block(b):
                nc.sync.dma_start(
                    kvr_sb[:, b * 1024:(b + 1) * 1024],
                    kvr[:, b * 1024:(b + 1) * 1024],
                )
                for c in range(4):
                    nc.sync.dma_start(
                        kvT_sb[c][:, b * 1024:(b + 1) * 1024],
                        kvT[c * 128:(c + 1) * 128, b * 1024:(b + 1) * 1024],
                    )

            qfT = []
            for h in range(HPC):
                qf = res.tile([128, NKC * T], BF16, tag=f"qft{h}")
                nc.sync.dma_start(qf[0:ROPE, 4 * T:5 * T], qpT[h])
                nc.sync.dma_start(qf[ROPE:128, 4 * T:5 * T], qpT[h])
                if h == 0:
                    load_kv_block(0)
                for c in range(4):
                    for tb in range(NTB):
                        ql_ps = ps_sc.tile([128, 512], FP32, tag="sc")
                        nc.tensor.matmul(
                            ql_ps[:],
                            wkc_sb[:, h, c * 128:(c + 1) * 128],
                            qnT_sb[:, h * T + tb * 512:h * T + (tb + 1) * 512],
                            kind="qlora",
                        )
                        nc.scalar.copy(
                            qf[:, c * T + tb * 512:c * T + (tb + 1) * 512], ql_ps[:]
                        )
                qfT.append(qf)

            # rest of the kv stream, s-block-major
            for hh in range(HPC):
                for cc in range(4):
                    nc.scalar.dma_start(wvc_sb[:, hh * 4 + cc, :], wvc[hh, cc])
            for b in range(1, 8):
                load_kv_block(b)

            # ---------------- main phases: (head, t-block)
            recip_sb = []
            attn_all = []
            for ph in range(HPC * NTB):
                h, tb = divmod(ph, NTB)
                attn_ps = ps_attn.tile([128, 4, 512], FP32, tag="attn")
                # p-sum accumulator in SBUF: the per-s denominator partial sums
                # run on DVE (otherwise idle) instead of 64 M=1 PE matmuls
                acc = smsb.tile([128, 512], FP32, tag="acc")
                pending = []  # (ss, kvl_ts, pts) of previous pairs: PV runs
                # two pairs behind so its exp dependency is long satisfied
                for sp in range(NST // 4):
                    ss = tuple(4 * sp + k for k in range(4))
                    kvl_ts = []
                    for s in ss:
                        kvl_t = kvlp.tile([128, LORA], BF16, tag="kvl", name="kvl_t")
                        nc.gpsimd.dma_start(kvl_t[:], kvl[s * 128:(s + 1) * 128, :])
                        kvl_ts.append(kvl_t)

                    # rope matmuls for 4 s-tiles back-to-back: K=64 each, in
                    # alternating PE row-groups -> pairs run concurrently and
                    # the row-group exit penalty is paid once per 4 tiles
                    scs = [ps_sc.tile([128, 512], FP32, tag="sc", name="sc_ps")
                           for _ in ss]
                    for i, s in enumerate(ss):
                        lo = (i % 2) * ROPE
                        nc.tensor.matmul(
                            scs[i][:],
                            kvr_sb[lo:lo + ROPE, s * 128:(s + 1) * 128],
                            qfT[h][lo:lo + ROPE, 4 * T + tb * 512:4 * T + (tb + 1) * 512],
                            kind="rope",
                            start=True,
                            stop=False,
                            tile_position=(lo, 0),
                        )
                    pts = []
                    for i, s in enumerate(ss):
                        for c in range(4):
                            nc.tensor.matmul(
                                scs[i][:],
                                kvT_sb[c][:, s * 128:(s + 1) * 128],
                                qfT[h][:, c * T + tb * 512:c * T + (tb + 1) * 512],
                                kind=f"score{c}",
                                start=False,
                                stop=(c == 3),
                            )
                        pt = ptp.tile([128, 512], BF16, tag="pt", name="pt")
                        nc.scalar.activation(pt[:], scs[i][:], Exp, scale=SCALING)
                        if s == 0:
                            nc.vector.tensor_copy(acc[:], pt[:])
                        else:
                            nc.vector.tensor_add(acc[:], acc[:], pt[:])
                        pts.append(pt)
                    def emit_pv(pv_ss, pv_kvl, pv_pts):
                        for i, s in enumerate(pv_ss):
                            for dt in range(4):
                                nc.tensor.matmul(
                                    attn_ps[:, dt, :],
                                    pv_kvl[i][:, dt * 128:(dt + 1) * 128],
                                    pv_pts[i][:],
                                    kind=f"pv{dt}",
                                    start=(s == 0),
                                    stop=(s == NST - 1),
                                )

                    pending.append((ss, kvl_ts, pts))
                    if len(pending) > 1:
                        emit_pv(*pending.pop(0))
                for p in pending:
                    emit_pv(*p)

                # phase epilogue: drain attn + denom, build 1/denomT [128t, 4]
                attn_sb = attnsb.tile([128, 4, 512], BF16, tag="attn")
                nc.scalar.copy(attn_sb[:], attn_ps[:])
                attn_all.append(attn_sb)

                den_ps = ps_sc.tile([1, 512], FP32, tag="sc", name="den_ps")
                nc.tensor.matmul(den_ps[:], ones_col[:], acc[:])
                den_sb = smsb.tile([1, 512], FP32, tag="den")
                nc.vector.tensor_copy(den_sb[:], den_ps[:])
                dT_ps = ps_sc.tile([128, 4], FP32, tag="sc", name="dT_ps")
                for j in range(4):
                    nc.tensor.matmul(
                        dT_ps[:, j:j + 1],
                        den_sb[0:1, j * 128:(j + 1) * 128],
                        ones_f32[0:1, 0:1],
                    )
                rc = smsb.tile([128, 4], FP32, tag="recip")
                nc.vector.reciprocal(rc[:], dT_ps[:])
                recip_sb.append(rc)

                # output projection out[t, v] = attnT.T @ w_vc, emitted per
                # phase so its PE work fills bubbles of the next phase
                for j in range(4):
                    op = ps_sc.tile([128, VDIM], FP32, tag="sc", name="op_ps")
                    for c in range(4):
                        nc.tensor.matmul(
                            op[:],
                            attn_sb[:, c, j * 128:(j + 1) * 128],
                            wvc_sb[:, h * 4 + c, :],
                            start=(c == 0),
                            stop=(c == 3),
                        )
                    ot = outsb.tile([128, VDIM], FP32, tag="out")
                    nc.vector.tensor_scalar_mul(ot[:], op[:], rc[:, j:j + 1])
                    nc.sync.dma_start(
                        out[tb * 512 + j * 128:tb * 512 + (j + 1) * 128,
                            h * VDIM:(h + 1) * VDIM],
                        ot[:],
                    )
    return nc


_PROGRAM = None


def _get_program():
    global _PROGRAM
    if _PROGRAM is None:
        _PROGRAM = _build_program()
    return _PROGRAM


# ---------------------------------------------------------------- host side
last_results = None  # BassKernelResults of the most recent run (for test.py)


def kernel(q, kv_cache, w_kc, w_vc):
    q = np.asarray(q, dtype=np.float32)
    kv_cache = np.asarray(kv_cache, dtype=np.float32)
    w_kc = np.asarray(w_kc, dtype=np.float32)
    w_vc = np.asarray(w_vc, dtype=np.float32)

    kvT_full = np.ascontiguousarray(kv_cache.T).astype(NPBF)       # [576, S]
    kvT_np = kvT_full[:LORA]                                        # [512, S]
    kvr_np = np.concatenate([kvT_full[LORA:], kvT_full[LORA:]], 0)  # [128, S] rope x2
    kvl_np = np.ascontiguousarray(kv_cache[:, :LORA]).astype(NPBF)  # [S, 512]

    in_maps = []
    for core in range(N_CORES):
        hs = [core * HPC + i for i in range(HPC)]
        qnT_np = np.stack(
            [np.ascontiguousarray(q[:, h, :NOPE].T) for h in hs]
        ).astype(NPBF)                                              # [HPC,128,T]
        qpT_np = np.stack(
            [np.ascontiguousarray(q[:, h, NOPE:].T) for h in hs]
        ).astype(NPBF)                                              # [HPC,64,T]
        wkc_np = np.ascontiguousarray(w_kc[hs]).astype(NPBF)        # [HPC,128,512]
        wvc_np = np.ascontiguousarray(
            w_vc[hs].reshape(HPC, 4, 128, VDIM)
        ).astype(NPBF)                                              # [HPC,4,128,128]
        in_maps.append(
            {
                "qnT": qnT_np,
                "qpT": qpT_np,
                "kvT": kvT_np,
                "kvr": kvr_np,
                "kvl": kvl_np,
                "wkc": wkc_np,
                "wvc": wvc_np,
            }
        )

    nc = _get_program()
    trace = bool(int(os.environ.get("KERNEL_TRACE", "0")))
    trace_cores = None
    if trace and os.environ.get("KERNEL_TRACE_CORES"):
        trace_cores = [
            int(x) for x in os.environ["KERNEL_TRACE_CORES"].split(",")
        ]
    res = run_bass_kernel_spmd(
        nc,
        in_maps,
        core_ids=list(range(N_CORES)),
        trace=trace,
        trace_cores=trace_cores,
    )
    global last_results
    last_results = res

    full = np.concatenate([res.results[c]["out"] for c in range(N_CORES)], axis=1)
    return np.ascontiguousarray(full.astype(np.float32))



# revision 2
# speedup vs baseline: 1.7379x; 1.7379x over previous
"""DeepseekV2 MLA (chunked-softmax MQA) on 8 trn2 NeuronCores.

Sharding: tensor-parallel over heads (16 heads / 8 cores = 2 heads per core);
the 576-wide latent KV cache is replicated per core. Each core computes its two
heads' full attention output [1024, 256]; the host concatenates along the
feature axis. All matmuls run in bf16 with fp32 PSUM accumulation.

Instead of the weight-absorbed form (score K=576, PV over the 512-wide
latent), each core un-absorbs the projections for its two heads up front:
  kT_h = w_kc[h] @ kv_loraT          (PE, [128 nope, S], N=512 matmuls)
  v    = kv_loraT.T @ [w_vc[h0]|w_vc[h1]]  ([128 s, 256] tiles, N=256)
so the inner loops shrink to
  scoresT = kT_h.T @ q_nopeT + ropeT.T @ q_peT   (K=128 + K=64 paired)
  pT      = exp(scale * scoresT)                  (ACT, PSUM->SBUF bf16)
  attnT  += v_tile.T @ pT                         (PE accum, [128 v, 512 t])
  out     = attnT.T via PE transpose, * 1/denom, DMA out.
This is ~2.4x less PE work than the absorbed form: 192-wide score
contraction instead of 576, 128-wide PV instead of 512, no output
projection (v is already projected).
"""

import os
import sys

import numpy as np
import ml_dtypes

for _p in ("/opt/trn_rl_repo",):
    if os.path.isdir(_p) and _p not in sys.path:
        sys.path.append(_p)

import concourse.bass as bass
import concourse.mybir as mybir
import concourse.tile as tile
from concourse.bass_utils import run_bass_kernel_spmd
from concourse.masks import make_identity
from concourse.vector_clock import ScopedClock, VectorClock

# ---------------------------------------------------------------- constants
NOPE, ROPE, LORA, VDIM = 128, 64, 512, 128
T, H, S = 1024, 16, 8192
D = LORA + ROPE            # 576 latent dim
SCALING = (NOPE + ROPE) ** -0.5
N_CORES = 8
HPC = H // N_CORES         # heads per core
NST = S // 128             # 64 s-tiles
NTB = T // 512             # 2 t-blocks
BF16 = mybir.dt.bfloat16
FP32 = mybir.dt.float32
NPBF = ml_dtypes.bfloat16


# ------------------------------------------------- walrus drain workaround
def _patch_tile_drain():
    """The neuronxcc walrus in this container rejects DRAIN instructions
    carrying more than ~2 sync waits ("Too many sync wait commands").
    Split the TileContext exit drain into one drain per processor tick;
    the waits execute sequentially on SP before the all-engine barrier,
    preserving the original semantics."""
    if getattr(tile.TileContext, "_drain_split_patched", False):
        return

    def _drain_and_barrier_split(self, tick_clock, wait_clock):
        gcv = tick_clock.global_clock
        n = len(gcv)
        for proc in range(n):
            t = gcv[proc]
            if t <= 0:
                continue
            vc = VectorClock([0] * n)
            vc.require_at_least(proc, t)
            d = self.nc.sync.drain()
            wait_clock.add_sem_waits(d.ins, ScopedClock({None: vc}))
        self.nc.all_engine_barrier()
        assert self.sems is not None
        popped = self.nc._tile_sem_poison_stack.pop()
        assert popped is self._sem_poison
        self.nc.clear_and_free_semaphores(list(self.sems.allocated().values()))
        self.nc.all_engine_barrier()

    tile.TileContext._drain_and_barrier = _drain_and_barrier_split

    # Same walrus limitation for regular instructions: peel all but the last
    # sync wait off onto same-engine NOPs inserted immediately before the
    # instruction. The engine executes its queue in order, so waiting on the
    # NOPs first is equivalent to one multi-wait instruction.
    orig_add = tile.TileContext._add_instruction

    def _add_instruction_split_waits(self, inst):
        si = inst.sync_info
        if si is not None:
            waits = si.on_wait
            if waits and len(waits) > 1:
                for w in waits[:-1]:
                    nop = mybir.InstNoOp(
                        name=self.nc.get_next_instruction_name(), ins=[], outs=[]
                    )
                    nop.engine = inst.engine
                    nop.sync_info = mybir.SyncInfo(on_wait=[w], on_update=[])
                    orig_add(self, nop)
                inst.sync_info = mybir.SyncInfo(
                    on_wait=[waits[-1]], on_update=si.on_update
                )
        orig_add(self, inst)

    tile.TileContext._add_instruction = _add_instruction_split_waits
    tile.TileContext._drain_split_patched = True


# ------------------------------------------------------------ bass program
MM_KINDS = {}


def _build_program():
    _patch_tile_drain()
    nc = bass.Bass()
    _orig_mm = nc.tensor.matmul

    def _mm_logged(out, lhsT, rhs, kind="?", **kw):
        inst = _orig_mm(out, lhsT, rhs, **kw)
        MM_KINDS[inst.ins.name] = kind
        return inst

    nc.tensor.matmul = _mm_logged
    qnT = nc.declare_dram_parameter("qnT", [HPC, NOPE, T], BF16, isOutput=False)
    qpT = nc.declare_dram_parameter("qpT", [HPC, ROPE, T], BF16, isOutput=False)
    kvT = nc.declare_dram_parameter("kvT", [LORA, S], BF16, isOutput=False)
    kvr = nc.declare_dram_parameter("kvr", [128, S], BF16, isOutput=False)
    wkcT = nc.declare_dram_parameter("wkcT", [HPC, 4, 128, NOPE], BF16, isOutput=False)
    wv2 = nc.declare_dram_parameter("wv2", [4, 128, HPC * VDIM], BF16, isOutput=False)
    out = nc.declare_dram_parameter("out", [T, HPC * VDIM], FP32, isOutput=True)

    Exp = mybir.ActivationFunctionType.Exp

    with tile.TileContext(nc) as tc:
        with (
            tc.tile_pool(name="res", bufs=1) as res,
            tc.tile_pool(name="ptp", bufs=8) as ptp,
            tc.tile_pool(name="attnsb", bufs=2) as attnsb,
            tc.tile_pool(name="smsb", bufs=4) as smsb,
            tc.tile_pool(name="outsb", bufs=3) as outsb,
            tc.tile_pool(name="ps_sc", bufs=4, space="PSUM") as ps_sc,
            tc.tile_pool(name="ps_kv", bufs=2, space="PSUM") as ps_kv,
            tc.tile_pool(name="ps_attn", bufs=1, space="PSUM") as ps_attn,
            tc.tile_pool(name="ps_out", bufs=1, space="PSUM") as ps_out,
        ):
            # ---------------- resident loads (small tensors first: they
            # unblock the k/v-gen matmuls that warm up PE while kvT streams)
            qnT_sb = res.tile([NOPE, HPC * T], BF16, tag="qnt")
            qpT_sb = res.tile([128, HPC * T], BF16, tag="qpt")
            for h in range(HPC):
                nc.sync.dma_start(qnT_sb[:, h * T:(h + 1) * T], qnT[h])
                # rope rows duplicated to partitions 64:128 so pairs of K=64
                # rope matmuls run concurrently in disjoint PE row-groups
                nc.scalar.dma_start(qpT_sb[0:ROPE, h * T:(h + 1) * T], qpT[h])
                nc.scalar.dma_start(qpT_sb[ROPE:128, h * T:(h + 1) * T], qpT[h])
            wkcT_sb = res.tile([128, HPC, 4, NOPE], BF16, tag="wkct")
            for h in range(HPC):
                for c in range(4):
                    nc.scalar.dma_start(wkcT_sb[:, h, c, :], wkcT[h, c])
            wv2_sb = res.tile([128, 4, HPC * VDIM], BF16, tag="wv2")
            for c in range(4):
                nc.scalar.dma_start(wv2_sb[:, c, :], wv2[c])

            ones_col = res.tile([128, 1], FP32, tag="ones_col")
            nc.vector.memset(ones_col[:], 1.0)
            ones_f32 = res.tile([1, 1], FP32, tag="ones_f32")
            nc.vector.memset(ones_f32[:], 1.0)
            ident = res.tile([128, 128], FP32, tag="ident")
            make_identity(nc, ident[:])

            # PE warmup: ~5us of matmuls on local data so HAM un-throttles and
            # the first real matmuls run at 2.4GHz instead of 1.2
            warm = res.tile([128, 512], BF16, tag="warm")
            nc.vector.memset(warm[:], 0.0)
            wu_ps = ps_sc.tile([128, 512], FP32, tag="sc", name="wu_ps")
            for _w in range(20):
                nc.tensor.matmul(wu_ps[:], warm[:, 0:128], warm[:, 0:512],
                                 kind="warm")

            # ---------------- kv stream
            kvT_sb = [
                res.tile([128, S], BF16, tag=f"kvt{c}", name=f"kvt{c}")
                for c in range(4)
            ]
            kvr_sb = res.tile([128, S], BF16, tag="kvr")

            def load_kv_block(b):
                nc.sync.dma_start(
                    kvr_sb[:, b * 1024:(b + 1) * 1024],
                    kvr[:, b * 1024:(b + 1) * 1024],
                )
                for c in range(4):
                    nc.sync.dma_start(
                        kvT_sb[c][:, b * 1024:(b + 1) * 1024],
                        kvT[c * 128:(c + 1) * 128, b * 1024:(b + 1) * 1024],
                    )

            for b in range(8):
                load_kv_block(b)

            # ---------------- k/v generation (un-absorbed projections)
            # kT_sb[h] [128 nope, S] = w_kc[h] @ kv_loraT
            # v_sb [128 s, 256] tiles = kv_loraT.T @ [w_vc[h0] | w_vc[h1]]
            kT_sb = [
                res.tile([NOPE, S], BF16, tag=f"kt{h}", name=f"kt{h}")
                for h in range(HPC)
            ]
            v_sb = res.tile([128, HPC * S], BF16, tag="vsb")
            for sb in range(16):
                sl = slice(sb * 512, (sb + 1) * 512)
                for h in range(HPC):
                    kps = ps_kv.tile([128, 512], FP32, tag="kv", name="kps")
                    for c in range(4):
                        nc.tensor.matmul(
                            kps[:],
                            wkcT_sb[:, h, c, :],
                            kvT_sb[c][:, sl],
                            kind="kgen",
                            start=(c == 0),
                            stop=(c == 3),
                        )
                    nc.scalar.copy(kT_sb[h][:, sl], kps[:])
                for st in range(4):
                    s = sb * 4 + st
                    vps = ps_kv.tile([128, HPC * VDIM], FP32, tag="kv",
                                     name="vps")
                    for c in range(4):
                        nc.tensor.matmul(
                            vps[:],
                            kvT_sb[c][:, s * 128:(s + 1) * 128],
                            wv2_sb[:, c, :],
                            kind="vgen",
                            start=(c == 0),
                            stop=(c == 3),
                        )
                    nc.vector.tensor_copy(
                        v_sb[:, s * (HPC * VDIM):(s + 1) * (HPC * VDIM)], vps[:]
                    )

            # ---------------- main phases: (head, t-block)
            for ph in range(HPC * NTB):
                h, tb = divmod(ph, NTB)
                tq = slice(h * T + tb * 512, h * T + (tb + 1) * 512)
                attn_ps = ps_attn.tile([128, 512], FP32, tag="attn")
                # p-sum accumulator in SBUF: the per-s denominator partial sums
                # run on DVE (otherwise idle) instead of M=1 PE matmuls
                acc = smsb.tile([128, 512], FP32, tag="acc")
                pending = []  # (ss, pts) of previous group: PV runs one
                # group behind so its exp dependency is long satisfied

                def emit_pv(pv_ss, pv_pts):
                    for i, s in enumerate(pv_ss):
                        nc.tensor.matmul(
                            attn_ps[:],
                            v_sb[:, s * 256 + h * VDIM:
                                 s * 256 + h * VDIM + VDIM],
                            pv_pts[i][:],
                            kind="pv",
                            start=(s == 0),
                            stop=(s == NST - 1),
                        )

                for sp in range(NST // 4):
                    ss = tuple(4 * sp + k for k in range(4))
                    # rope matmuls for 4 s-tiles back-to-back: K=64 each, in
                    # alternating PE row-groups -> pairs run concurrently and
                    # the row-group exit penalty is paid once per 4 tiles
                    scs = [ps_sc.tile([128, 512], FP32, tag="sc", name="sc_ps")
                           for _ in ss]
                    for i, s in enumerate(ss):
                        lo = (i % 2) * ROPE
                        nc.tensor.matmul(
                            scs[i][:],
                            kvr_sb[lo:lo + ROPE, s * 128:(s + 1) * 128],
                            qpT_sb[lo:lo + ROPE, tq],
                            kind="rope",
                            start=True,
                            stop=False,
                            tile_position=(lo, 0),
                        )
                    pts = []
                    for i, s in enumerate(ss):
                        nc.tensor.matmul(
                            scs[i][:],
                            kT_sb[h][:, s * 128:(s + 1) * 128],
                            qnT_sb[:, tq],
                            kind="nope",
                            start=False,
                            stop=True,
                        )
                        pt = ptp.tile([128, 512], BF16, tag="pt", name="pt")
                        nc.scalar.activation(pt[:], scs[i][:], Exp,
                                             scale=SCALING)
                        if s == 0:
                            nc.vector.tensor_copy(acc[:], pt[:])
                        else:
                            nc.vector.tensor_add(acc[:], acc[:], pt[:])
                        pts.append(pt)

                    pending.append((ss, pts))
                    if len(pending) > 1:
                        emit_pv(*pending.pop(0))
                for p in pending:
                    emit_pv(*p)

                # phase epilogue: drain attn + denom, build 1/denomT [128t, 4]
                attn_sb = attnsb.tile([128, 512], FP32, tag="attn")
                nc.scalar.copy(attn_sb[:], attn_ps[:])

                den_ps = ps_sc.tile([1, 512], FP32, tag="sc", name="den_ps")
                nc.tensor.matmul(den_ps[:], ones_col[:], acc[:])
                den_sb = smsb.tile([1, 512], FP32, tag="den")
                nc.vector.tensor_copy(den_sb[:], den_ps[:])
                dT_ps = ps_sc.tile([128, 4], FP32, tag="sc", name="dT_ps")
                for j in range(4):
                    nc.tensor.matmul(
                        dT_ps[:, j:j + 1],
                        den_sb[0:1, j * 128:(j + 1) * 128],
                        ones_f32[0:1, 0:1],
                    )
                rc = smsb.tile([128, 4], FP32, tag="recip")
                nc.vector.reciprocal(rc[:], dT_ps[:])

                # output: PE-transpose attnT [128 v, 512 t] into [t, v]
                # tiles, scale by 1/denom (per-partition scalar), DMA out
                for j in range(4):
                    tp = ps_out.tile([128, 128], FP32, tag="out", name="tp")
                    nc.tensor.transpose(
                        tp[:], attn_sb[:, j * 128:(j + 1) * 128], ident[:]
                    )
                    ot = outsb.tile([128, VDIM], FP32, tag="out")
                    nc.vector.tensor_scalar_mul(ot[:], tp[:], rc[:, j:j + 1])
                    nc.sync.dma_start(
                        out[tb * 512 + j * 128:tb * 512 + (j + 1) * 128,
                            h * VDIM:(h + 1) * VDIM],
                        ot[:],
                    )
    return nc


_PROGRAM = None


def _get_program():
    global _PROGRAM
    if _PROGRAM is None:
        _PROGRAM = _build_program()
    return _PROGRAM


# ---------------------------------------------------------------- host side
last_results = None  # BassKernelResults of the most recent run (for test.py)


def kernel(q, kv_cache, w_kc, w_vc):
    q = np.asarray(q, dtype=np.float32)
    kv_cache = np.asarray(kv_cache, dtype=np.float32)
    w_kc = np.asarray(w_kc, dtype=np.float32)
    w_vc = np.asarray(w_vc, dtype=np.float32)

    kvT_full = np.ascontiguousarray(kv_cache.T).astype(NPBF)       # [576, S]
    kvT_np = kvT_full[:LORA]                                        # [512, S]
    kvr_np = np.concatenate([kvT_full[LORA:], kvT_full[LORA:]], 0)  # [128, S] rope x2

    in_maps = []
    for core in range(N_CORES):
        hs = [core * HPC + i for i in range(HPC)]
        qnT_np = np.stack(
            [np.ascontiguousarray(q[:, h, :NOPE].T) for h in hs]
        ).astype(NPBF)                                              # [HPC,128,T]
        qpT_np = np.stack(
            [np.ascontiguousarray(q[:, h, NOPE:].T) for h in hs]
        ).astype(NPBF)                                              # [HPC,64,T]
        # wkcT[h, c, l, n] = w_kc[h, n, c*128+l]  (lhsT for k-gen)
        wkcT_np = np.ascontiguousarray(
            w_kc[hs].transpose(0, 2, 1).reshape(HPC, 4, 128, NOPE)
        ).astype(NPBF)
        # wv2[c, l, h*128+v] = w_vc[h, c*128+l, v]  (rhs for v-gen, both heads)
        wv2_np = np.ascontiguousarray(
            w_vc[hs].transpose(1, 0, 2).reshape(4, 128, HPC * VDIM)
        ).astype(NPBF)
        in_maps.append(
            {
                "qnT": qnT_np,
                "qpT": qpT_np,
                "kvT": kvT_np,
                "kvr": kvr_np,
                "wkcT": wkcT_np,
                "wv2": wv2_np,
            }
        )

    nc = _get_program()
    trace = bool(int(os.environ.get("KERNEL_TRACE", "0")))
    trace_cores = None
    if trace and os.environ.get("KERNEL_TRACE_CORES"):
        trace_cores = [
            int(x) for x in os.environ["KERNEL_TRACE_CORES"].split(",")
        ]
    res = run_bass_kernel_spmd(
        nc,
        in_maps,
        core_ids=list(range(N_CORES)),
        trace=trace,
        trace_cores=trace_cores,
    )
    global last_results
    last_results = res

    full = np.concatenate([res.results[c]["out"] for c in range(N_CORES)], axis=1)
    return np.ascontiguousarray(full.astype(np.float32))
